# revision 56
# baseline (speedup 1.0000x reference)
"""Trainium2 kernel for nn_PositionalEncodingQuantum.

Math: reference output is out[0, t, e] = cos(t) * Wsum[e] + b[e] where
Wsum[e] = sum_q W[e, q]  (pe[b,t,q] = cos(t) for every q; batch dim
collapses to 1). Pure HBM-write-bound: 65536 x 1024 f32 = 256 MB out.

Sharding: T split across 8 cores (8192 rows each). Host precomputes
Wsum, and cos(t) for each core's rows in f64 (cheap: 64K values), and
ships one pre-broadcast [128, 2112] tensor per core:
  cols 0..1023    Wsum broadcast to all 128 partitions
  cols 1024..2047 b broadcast
  cols 2048..2111 cos_t block: col 2048+blk holds cos(t0 + 128*blk + p)
Device: one VectorE scalar_tensor_tensor per 128x1024 output block
(out = cos_t[p,blk]*Wsum + b), DMA'd out in 8 ramped groups.
"""

import numpy as np

EMBED = 1024
T = 65536
NCORES = 8
T_CORE = T // NCORES          # 8192 rows per core
NBLK = T_CORE // 128          # 64 blocks of [128, 1024]
# Ramped DMA groups: first out-DMA fires after 1 STT (~1us) instead of 8
# (~9us), shaving pipeline fill; still exactly 8 out-DMAs so the 8 SWDGE
# sems are used once each (sole reuse: DMASW0, dominated by pool_pre).
SIZES = [1, 7, 8, 8, 8, 8, 8, 16]   # blocks per group, sum = NBLK
RING = 32                            # SBUF ring capacity in blocks (128KB/part)
WTB_W = 2 * EMBED + NBLK      # 2112

_CACHE = {}


def _drain_via_sp_chain(self, tick_clock, wait_clock):
    # Replacement for TileContext._drain_and_barrier. The stock version
    # attaches one sem-wait per live semaphore to the final Drain, but
    # codegen caps sync waits per instruction; with 10+ sem families the
    # Drain fails to lower. Instead, emit one wait_ge per semaphore on the
    # SP queue (each a separate 1-wait instruction, SP is in-order), after
    # which a bare Drain is equivalent.
    from collections import defaultdict

    upd = defaultdict(int)
    for insts in self.ordered_instructions_by_block.values():
        for i in insts:
            si = getattr(i, "sync_info", None)
            if si is None:
                continue
            for u in si.on_update or []:
                assert u.update_mode in ("sem-add-imm", "sem-inc"), u
                upd[u.ant_name] += u.update_value if u.update_mode == "sem-add-imm" else 1
    handles = {h.name: h for h in self.sems.allocated().values()}
    nc = self.nc
    for name, val in upd.items():
        nc.sync.wait_ge(handles[name], val)
    nc.sync.drain()
    nc.all_engine_barrier()
    popped = nc._tile_sem_poison_stack.pop()
    assert popped is self._sem_poison
    nc.clear_and_free_semaphores(list(self.sems.allocated().values()))
    nc.all_engine_barrier()


def _build_module():
    import concourse.bass as bass
    import concourse.tile as tile
    from concourse import mybir

    tile.TileContext._drain_and_barrier = _drain_via_sp_chain

    f32 = mybir.dt.float32
    nc = bass.Bass()
    wtb_d = nc.declare_dram_parameter("wtb", [128, WTB_W], f32, isOutput=False)
    out_d = nc.declare_dram_parameter("out", [T_CORE, EMBED], f32, isOutput=True)

    add = mybir.AluOpType.add
    mult = mybir.AluOpType.mult

    with tile.TileContext(nc) as tc:
        with (
            tc.tile_pool(name="const", bufs=1) as const,
            tc.tile_pool(name="work", bufs=1) as work,
        ):
            # Codegen allows a single sync-wait per TPB compute / DMA
            # descriptor. One SWDGE input DMA (sem DMASW0) feeds every DVE
            # STT: the first STT waits DMASW0>=16, later DVE waits elide.
            wtb_raw = const.tile([128, WTB_W], f32)
            nc.gpsimd.dma_start(out=wtb_raw[:], in_=wtb_d[:])
            # DMASW0 is reused by the last out-DMA; this Pool copy consumes
            # DMASW0>=16 early on the Pool queue so that trigger's reuse
            # wait is dominated (trigger then carries only its DVE wait).
            pool_pre = const.tile([1, 1], f32)
            nc.gpsimd.tensor_copy(pool_pre[:], wtb_raw[0:1, 0:1])

            wsum_b = wtb_raw[:, 0:EMBED]
            b_b = wtb_raw[:, EMBED : 2 * EMBED]

            # ---- main loop: out[128*blk + p, e] = cos_t[p, blk]*Wsum[e] + b[e]
            # p (partition) must stay outermost on both sides of the DMA:
            # demoting it in the SBUF-side pattern silently drops the
            # partition stride and scrambles the transfer.
            ov = out_d.rearrange("(t p) e -> p t e", p=128)
            # One persistent 16 MB tile used as a 32-block ring, rotated
            # manually by group size. Within a single tile handle's lifetime
            # Tile tracks deps region-granularly, so a block's new writer
            # deps only on that block's old accessors (unlike pool
            # recycling, which is tile-coarse). After each group, a tiny DVE
            # "fence" copy reads the group's last column: it is ready the
            # moment the group's STTs finish (a full DMA earlier than any
            # dependent), so the scheduler always queues it before the
            # recycling group, keeping every recycled STT at exactly one
            # wait (WAR on the old out-DMA's completion sem).
            big = work.tile([128, RING * EMBED], f32)
            scratch = const.tile([128, 1], f32)
            blk0 = 0
            off = 0
            for n in SIZES:
                if off + n > RING:
                    off = 0
                base = off * EMBED
                for j in range(n):
                    blk = blk0 + j
                    nc.vector.scalar_tensor_tensor(
                        big[:, base + EMBED * j : base + EMBED * (j + 1)],
                        wsum_b,
                        wtb_raw[:, 2 * EMBED + blk : 2 * EMBED + blk + 1],
                        b_b,
                        mult,
                        add,
                    )
                seg = big[:, base : base + n * EMBED]
                nc.gpsimd.dma_start(
                    out=ov[:, blk0 : blk0 + n, :],
                    in_=seg.rearrange("p (j e) -> p j e", j=n),
                )
                nc.vector.tensor_copy(
                    scratch[:], big[:, base + n * EMBED - 1 : base + n * EMBED]
                )
                blk0 += n
                off = (off + n) % RING
    return nc


def _make_in_maps(W: np.ndarray, b: np.ndarray):
    import jax.numpy as jnp

    wsum = W.astype(np.float32).sum(axis=1)  # [1024]
    base = np.zeros((128, WTB_W), dtype=np.float32)
    base[:, :EMBED] = wsum[None, :]
    base[:, EMBED : 2 * EMBED] = b.astype(np.float32)[None, :]
    # Must match the reference's jnp.cos(f32) bit-for-bit: XLA's f32 cos
    # range reduction differs from an exact f64 cos by up to ~4e-3 at
    # t ~ 65535, which would dominate the error budget.
    cos_full = np.asarray(jnp.cos(jnp.arange(T, dtype=jnp.float32)))
    in_maps = []
    for c in range(NCORES):
        wtb = base.copy()
        # cos_t[p, blk] = cos(T_CORE*c + 128*blk + p)
        wtb[:, 2 * EMBED :] = cos_full[T_CORE * c : T_CORE * (c + 1)].reshape(NBLK, 128).T
        in_maps.append({"wtb": wtb})
    return in_maps


def kernel(x: np.ndarray, W: np.ndarray, b: np.ndarray) -> np.ndarray:
    from concourse.bass_utils import run_bass_kernel_spmd

    if "nc" not in _CACHE:
        _CACHE["nc"] = _build_module()
    nc = _CACHE["nc"]

    res = run_bass_kernel_spmd(nc, _make_in_maps(W, b), list(range(NCORES)))
    shards = [res.results[c]["out"] for c in range(NCORES)]
    full = np.concatenate(shards, axis=0)  # [65536, 1024]
    return full[None, :, :].astype(np.float32, copy=False)


# revision 58
# speedup vs baseline: 1.2452x; 1.2452x over previous
"""Trainium2 kernel for nn_PositionalEncodingQuantum.

Math: reference output is out[0, t, e] = cos(t) * Wsum[e] + b[e] where
Wsum[e] = sum_q W[e, q]  (pe[b,t,q] = cos(t) for every q; batch dim
collapses to 1). Pure HBM-write-bound: 65536 x 1024 f32 = 256 MB out.

Sharding: T split across 8 cores (8192 rows each). Host precomputes
Wsum, and cos(t) for each core's rows in f64 (cheap: 64K values), and
ships one pre-broadcast [128, 2112] tensor per core:
  cols 0..1023    Wsum broadcast to all 128 partitions
  cols 1024..2047 b broadcast
  cols 2048..2111 cos_t block: col 2048+blk holds cos(t0 + 128*blk + p)
Device: one VectorE scalar_tensor_tensor per 128x1024 output block
(out = cos_t[p,blk]*Wsum + b), DMA'd out in 8 ramped groups.
"""

import numpy as np

EMBED = 1024
T = 65536
NCORES = 8
T_CORE = T // NCORES          # 8192 rows per core
NBLK = T_CORE // 128          # 64 blocks of [128, 1024]
# Ramped DMA groups: first out-DMA fires after 1 STT (~1us) instead of 8
# (~9us), shaving pipeline fill; still exactly 8 out-DMAs so the 8 SWDGE
# sems are used once each (sole reuse: DMASW0, dominated by pool_pre).
SIZES = [1, 7, 8, 8, 8, 8, 8, 16]   # blocks per group, sum = NBLK
RING = 32                            # SBUF ring capacity in blocks (128KB/part)
WTB_W = 2 * EMBED + NBLK      # 2112

_CACHE = {}


def _drain_via_sp_chain(self, tick_clock, wait_clock):
    # Replacement for TileContext._drain_and_barrier. The stock version
    # attaches one sem-wait per live semaphore to the final Drain, but
    # codegen caps sync waits per instruction; with 10+ sem families the
    # Drain fails to lower. Instead, emit one wait_ge per semaphore on the
    # SP queue (each a separate 1-wait instruction, SP is in-order), after
    # which a bare Drain is equivalent.
    from collections import defaultdict

    upd = defaultdict(int)
    for insts in self.ordered_instructions_by_block.values():
        for i in insts:
            si = getattr(i, "sync_info", None)
            if si is None:
                continue
            for u in si.on_update or []:
                assert u.update_mode in ("sem-add-imm", "sem-inc"), u
                upd[u.ant_name] += u.update_value if u.update_mode == "sem-add-imm" else 1
    handles = {h.name: h for h in self.sems.allocated().values()}
    nc = self.nc
    for name, val in upd.items():
        nc.sync.wait_ge(handles[name], val)
    nc.sync.drain()
    nc.all_engine_barrier()
    popped = nc._tile_sem_poison_stack.pop()
    assert popped is self._sem_poison
    nc.clear_and_free_semaphores(list(self.sems.allocated().values()))
    nc.all_engine_barrier()


def _build_module():
    import concourse.bass as bass
    import concourse.tile as tile
    from concourse import mybir

    tile.TileContext._drain_and_barrier = _drain_via_sp_chain

    f32 = mybir.dt.float32
    nc = bass.Bass()
    wtb_d = nc.declare_dram_parameter("wtb", [128, WTB_W], f32, isOutput=False)
    out_d = nc.declare_dram_parameter("out", [T_CORE, EMBED], f32, isOutput=True)

    add = mybir.AluOpType.add
    mult = mybir.AluOpType.mult

    with tile.TileContext(nc) as tc:
        with (
            tc.tile_pool(name="const", bufs=1) as const,
            tc.tile_pool(name="work", bufs=1) as work,
        ):
            # Codegen allows a single sync-wait per TPB compute / DMA
            # descriptor. One SWDGE input DMA (sem DMASW0) feeds every DVE
            # STT: the first STT waits DMASW0>=16, later DVE waits elide.
            wtb_raw = const.tile([128, WTB_W], f32)
            nc.gpsimd.dma_start(out=wtb_raw[:], in_=wtb_d[:])
            # DMASW0 is reused by the last out-DMA; this Pool copy consumes
            # DMASW0>=16 early on the Pool queue so that trigger's reuse
            # wait is dominated (trigger then carries only its DVE wait).
            pool_pre = const.tile([1, 1], f32)
            nc.gpsimd.tensor_copy(pool_pre[:], wtb_raw[0:1, 0:1])

            wsum_b = wtb_raw[:, 0:EMBED]
            b_b = wtb_raw[:, EMBED : 2 * EMBED]

            # ---- main loop: out[128*blk + p, e] = cos_t[p, blk]*Wsum[e] + b[e]
            # p (partition) must stay outermost on both sides of the DMA:
            # demoting it in the SBUF-side pattern silently drops the
            # partition stride and scrambles the transfer.
            # row = p*64 + t: partition p owns 64 consecutive output rows, so
            # each group's HBM write is one n*4KB contiguous burst per
            # partition (vs 4KB bursts with the t-interleaved layout).
            ov = out_d.rearrange("(p t) e -> p t e", p=128)
            # One persistent 16 MB tile used as a 32-block ring, rotated
            # manually by group size. Within a single tile handle's lifetime
            # Tile tracks deps region-granularly, so a block's new writer
            # deps only on that block's old accessors (unlike pool
            # recycling, which is tile-coarse). After each group, a tiny DVE
            # "fence" copy reads the group's last column: it is ready the
            # moment the group's STTs finish (a full DMA earlier than any
            # dependent), so the scheduler always queues it before the
            # recycling group, keeping every recycled STT at exactly one
            # wait (WAR on the old out-DMA's completion sem).
            big = work.tile([128, RING * EMBED], f32)
            scratch = const.tile([128, 1], f32)
            blk0 = 0
            off = 0
            for n in SIZES:
                if off + n > RING:
                    off = 0
                base = off * EMBED
                for j in range(n):
                    blk = blk0 + j
                    nc.vector.scalar_tensor_tensor(
                        big[:, base + EMBED * j : base + EMBED * (j + 1)],
                        wsum_b,
                        wtb_raw[:, 2 * EMBED + blk : 2 * EMBED + blk + 1],
                        b_b,
                        mult,
                        add,
                    )
                seg = big[:, base : base + n * EMBED]
                nc.gpsimd.dma_start(
                    out=ov[:, blk0 : blk0 + n, :],
                    in_=seg.rearrange("p (j e) -> p j e", j=n),
                )
                nc.vector.tensor_copy(
                    scratch[:], big[:, base + n * EMBED - 1 : base + n * EMBED]
                )
                blk0 += n
                off = (off + n) % RING
    return nc


def _make_in_maps(W: np.ndarray, b: np.ndarray):
    import jax.numpy as jnp

    wsum = W.astype(np.float32).sum(axis=1)  # [1024]
    base = np.zeros((128, WTB_W), dtype=np.float32)
    base[:, :EMBED] = wsum[None, :]
    base[:, EMBED : 2 * EMBED] = b.astype(np.float32)[None, :]
    # Must match the reference's jnp.cos(f32) bit-for-bit: XLA's f32 cos
    # range reduction differs from an exact f64 cos by up to ~4e-3 at
    # t ~ 65535, which would dominate the error budget.
    cos_full = np.asarray(jnp.cos(jnp.arange(T, dtype=jnp.float32)))
    in_maps = []
    for c in range(NCORES):
        wtb = base.copy()
        # cos_t[p, blk] = cos(T_CORE*c + NBLK*p + blk)  (row = p*64 + t layout)
        wtb[:, 2 * EMBED :] = cos_full[T_CORE * c : T_CORE * (c + 1)].reshape(128, NBLK)
        in_maps.append({"wtb": wtb})
    return in_maps


def kernel(x: np.ndarray, W: np.ndarray, b: np.ndarray) -> np.ndarray:
    from concourse.bass_utils import run_bass_kernel_spmd

    if "nc" not in _CACHE:
        _CACHE["nc"] = _build_module()
    nc = _CACHE["nc"]

    res = run_bass_kernel_spmd(nc, _make_in_maps(W, b), list(range(NCORES)))
    shards = [res.results[c]["out"] for c in range(NCORES)]
    full = np.concatenate(shards, axis=0)  # [65536, 1024]
    return full[None, :, :].astype(np.float32, copy=False)
